# revision 15
# baseline (speedup 1.0000x reference)
"""Bilateral slice-apply kernel for Trainium2 (8 NeuronCores, SPMD).

Sharding: core = (batch b = core//2) x (H-half = core%2); each core handles
512 rows x 1024 cols of one batch. The tiny grid is preprocessed on host into
per-row y-interpolated ramp-basis difference tables, so the device-side
z-interpolation is a dense sum of clamped ramps (no gathers):

  zeval(gz) = T0 + sum_{k=1..7} (T_k - T_{k-1}) * clamp01(gz - (k - 0.5))

which is exact for tent-weight trilinear slicing with clamped borders.
x-interpolation uses the 32px-block structure (grid-col pair constant per
block) with broadcast access patterns; the affine apply is done per pixel.
"""

import numpy as np
from contextlib import ExitStack

import concourse.bass as bass
import concourse.bacc as bacc
import concourse.tile as tile
from concourse import mybir
from concourse.bass_utils import run_bass_kernel_spmd

f32 = mybir.dt.float32
OP = mybir.AluOpType

# hardcoded problem shapes
B, C, GD, GH, GW = 4, 12, 8, 16, 16
H, W = 1024, 1024
NCORES = 8
RH = H // 2           # rows per core
NRG = RH // 128       # rowgroups per core
NM = W // 32          # 32px x-blocks across full W
XH = W // 2           # x half-width processed per inner step
MH = NM // 2          # x-blocks per half


def _ap(base, free):
    """AP with base's partition dim and explicit free dims."""
    return bass.AP(tensor=base.tensor, offset=base.offset, ap=[base.ap[0]] + free)


def _build_nc():
    nc = bacc.Bacc("TRN2", target_bir_lowering=False, debug=False,
                   num_devices=NCORES)

    guide_in = nc.dram_tensor("guide", [RH, W], f32, kind="ExternalInput")
    img_in = nc.dram_tensor("image", [3, RH, W], f32, kind="ExternalInput")
    tab_in = nc.dram_tensor("tab", [RH, C, 8, 2, 32], f32, kind="ExternalInput")
    wxr_in = nc.dram_tensor("wxr", [W], f32, kind="ExternalInput")
    km_in = nc.dram_tensor("km", [7], f32, kind="ExternalInput")
    out_t = nc.dram_tensor("out", [3, RH, W], f32, kind="ExternalOutput")

    with tile.TileContext(nc) as tc, ExitStack() as ctx:
        consts = ctx.enter_context(tc.tile_pool(name="consts", bufs=1))
        tabp = ctx.enter_context(tc.tile_pool(name="tabp", bufs=2))
        pix = ctx.enter_context(tc.tile_pool(name="pix", bufs=2))
        work = ctx.enter_context(tc.tile_pool(name="work", bufs=1))
        rpool = ctx.enter_context(tc.tile_pool(name="rpool", bufs=1))
        outp = ctx.enter_context(tc.tile_pool(name="outp", bufs=2))

        # one-time partition-broadcast constants
        wxr = consts.tile([128, W], f32)
        nc.sync.dma_start(
            out=wxr,
            in_=bass.AP(tensor=wxr_in[:].tensor, offset=wxr_in[:].offset,
                        ap=[[0, 128]] + wxr_in[:].ap),
        )
        km = consts.tile([128, 7], f32)
        nc.sync.dma_start(
            out=km,
            in_=bass.AP(tensor=km_in[:].tensor, offset=km_in[:].offset,
                        ap=[[0, 128]] + km_in[:].ap),
        )

        for rg in range(NRG):
            r0 = rg * 128
            gt = pix.tile([128, W], f32, tag="guide")
            nc.sync.dma_start(out=gt, in_=guide_in[r0:r0 + 128, :])
            tabt = tabp.tile([128, C, 8, 2, 32], f32, tag="tab")
            nc.sync.dma_start(out=tabt, in_=tab_in[r0:r0 + 128])
            imgs = []
            for j in range(3):
                it = pix.tile([128, W], f32, tag=f"img{j}")
                nc.sync.dma_start(out=it, in_=img_in[j, r0:r0 + 128, :])
                imgs.append(it)

            for xh in range(2):
                x0 = xh * XH
                # ramps r_k = clamp01(8*guide - (k-0.5)), k=1..7 -> [128,7,XH]
                rr = rpool.tile([128, 7, XH], f32, tag="ramps")
                g2 = gt[:, x0:x0 + XH]
                nc.vector.scalar_tensor_tensor(
                    out=rr,
                    in0=_ap(g2, [[0, 7]] + g2.ap[1:]),
                    scalar=8.0,
                    in1=_ap(km[:, :], [km[:, :].ap[1], [0, XH]]),
                    op0=OP.mult, op1=OP.subtract,
                )
                nc.vector.tensor_scalar(
                    out=rr, in0=rr, scalar1=0.0, scalar2=1.0,
                    op0=OP.max, op1=OP.min,
                )

                # channel groups: c0..7 on DVE, c8..11 on GPSIMD (~2x slower)
                groups = [(0, 8, nc.vector), (8, 4, nc.gpsimd)]
                coeffs = []
                for c0c, ng, eng in groups:
                    accB = work.tile([128, ng, MH, 32], f32, tag=f"accB{c0c}")
                    accD = work.tile([128, ng, MH, 32], f32, tag=f"accD{c0c}")
                    tmp = work.tile([128, ng, MH, 32], f32, tag=f"tmp{c0c}")

                    # init from k=0 slot (AL, AD), broadcast 32x along u (ACT)
                    for s, acc in ((0, accB), (1, accD)):
                        t5 = tabt[:, c0c:c0c + ng, 0, s, xh * MH:(xh + 1) * MH]
                        nc.scalar.copy(
                            out=acc, in_=_ap(t5, t5.ap[1:] + [[0, 32]])
                        )
                    for k in range(1, 8):
                        rk = rr[:, k - 1, :]
                        rk_b = _ap(rk, [[0, ng], [32, MH], [1, 32]])
                        for s, acc in ((0, accB), (1, accD)):
                            t5 = tabt[:, c0c:c0c + ng, k, s, xh * MH:(xh + 1) * MH]
                            eng.tensor_tensor(
                                out=tmp, in0=rk_b,
                                in1=_ap(t5, t5.ap[1:] + [[0, 32]]),
                                op=OP.mult,
                            )
                            eng.tensor_tensor(out=acc, in0=acc, in1=tmp,
                                              op=OP.add)
                    # coeff = accB + wxr * accD
                    w2 = wxr[:, x0:x0 + XH]
                    eng.tensor_tensor(
                        out=tmp, in0=accD,
                        in1=_ap(w2, [[0, ng], [32, MH], [1, 32]]),
                        op=OP.mult,
                    )
                    eng.tensor_tensor(out=accB, in0=accB, in1=tmp, op=OP.add)
                    coeffs.append(accB)

                # affine apply: out_i = c4i*R + c4i+1*G + c4i+2*B + c4i+3
                def cslice(c):
                    gidx = 0 if c < 8 else 1
                    return coeffs[gidx][:, c - 8 * gidx]  # [128, MH, 32]

                ims = [
                    _ap(imgs[j][:, x0:x0 + XH], [[32, MH], [1, 32]])
                    for j in range(3)
                ]
                for i in range(3):
                    eng = nc.vector if i < 2 else nc.gpsimd
                    t0 = work.tile([128, MH, 32], f32, tag=f"t0_{i}")
                    t1 = work.tile([128, MH, 32], f32, tag=f"t1_{i}")
                    cs = [cslice(4 * i + j) for j in range(4)]
                    eng.tensor_tensor(out=t0, in0=cs[0], in1=ims[0], op=OP.mult)
                    eng.tensor_tensor(out=t1, in0=cs[1], in1=ims[1], op=OP.mult)
                    eng.tensor_tensor(out=t0, in0=t0, in1=t1, op=OP.add)
                    eng.tensor_tensor(out=t1, in0=cs[2], in1=ims[2], op=OP.mult)
                    eng.tensor_tensor(out=t0, in0=t0, in1=t1, op=OP.add)
                    ot = outp.tile([128, MH, 32], f32, tag=f"o{i}")
                    eng.tensor_tensor(out=ot, in0=t0, in1=cs[3], op=OP.add)
                    nc.sync.dma_start(
                        out=out_t[i, r0:r0 + 128, x0:x0 + XH],
                        in_=_ap(ot[:, :], [[1, XH]]),
                    )
    nc.compile()
    return nc


_NC = None


def _get_nc():
    global _NC
    if _NC is None:
        _NC = _build_nc()
    return _NC


def _host_tables(grid_b, row0):
    """Build per-row ramp-basis tables [RH, C, 8, 2, 32] for rows
    [row0, row0+RH) of one batch's grid [C, GD, GH, GW]."""
    g = grid_b.astype(np.float32)
    y = np.arange(row0, row0 + RH, dtype=np.float64)
    gy = (y + 0.5) * GH / H
    fy = np.floor(gy - 0.5)
    wy1 = (gy - 0.5 - fy).astype(np.float32)
    iy0 = np.clip(fy, 0, GH - 1).astype(np.int64)
    iy1 = np.clip(fy + 1, 0, GH - 1).astype(np.int64)

    # F: [C, GD, RH, GW]
    F = (1.0 - wy1)[None, None, :, None] * g[:, :, iy0, :] \
        + wy1[None, None, :, None] * g[:, :, iy1, :]
    A = F[:, 0]                         # [C, RH, GW]
    D = F[:, 1:] - F[:, :-1]            # [C, 7, RH, GW]

    x = np.arange(W, dtype=np.float64)
    gx = (x + 0.5) * GW / W
    fx = np.floor(gx - 0.5)
    ix0 = np.clip(fx, 0, GW - 1).astype(np.int64).reshape(NM, 32)[:, 0]
    ix1 = np.clip(fx + 1, 0, GW - 1).astype(np.int64).reshape(NM, 32)[:, 0]

    tab = np.empty((RH, C, 8, 2, 32), np.float32)
    AL, AR = A[:, :, ix0], A[:, :, ix1]              # [C, RH, NM]
    tab[:, :, 0, 0, :] = AL.transpose(1, 0, 2)
    tab[:, :, 0, 1, :] = (AR - AL).transpose(1, 0, 2)
    DL, DR = D[:, :, :, ix0], D[:, :, :, ix1]        # [C, 7, RH, NM]
    tab[:, :, 1:, 0, :] = DL.transpose(2, 0, 1, 3)
    tab[:, :, 1:, 1, :] = (DR - DL).transpose(2, 0, 1, 3)
    return np.ascontiguousarray(tab)


def _host_wxr():
    x = np.arange(W, dtype=np.float64)
    gx = (x + 0.5) * GW / W
    fx = np.floor(gx - 0.5)
    return (gx - 0.5 - fx).astype(np.float32)


def _make_in_maps(grid, guide, image):
    wxr = _host_wxr()
    km = (np.arange(1, 8) - 0.5).astype(np.float32)
    in_maps = []
    for core in range(NCORES):
        b, hh = core // 2, core % 2
        r0 = hh * RH
        in_maps.append({
            "guide": np.ascontiguousarray(guide[b, r0:r0 + RH]).astype(np.float32),
            "image": np.ascontiguousarray(image[b, :, r0:r0 + RH]).astype(np.float32),
            "tab": _host_tables(grid[b], r0),
            "wxr": wxr,
            "km": km,
        })
    return in_maps


def _run(grid, guide, image, trace=False):
    nc = _get_nc()
    in_maps = _make_in_maps(grid, guide, image)
    res = run_bass_kernel_spmd(nc, in_maps, core_ids=list(range(NCORES)),
                               trace=trace)
    out = np.empty((B, 3, H, W), np.float32)
    for core in range(NCORES):
        b, hh = core // 2, core % 2
        out[b, :, hh * RH:hh * RH + RH, :] = res.results[core]["out"]
    return out, res


def kernel(grid, guide, image):
    out, _ = _run(grid, guide, image, trace=False)
    return out


# revision 19
# speedup vs baseline: 1.0308x; 1.0308x over previous
"""Bilateral slice-apply kernel for Trainium2 (8 NeuronCores, SPMD).

Sharding: core = (batch b = core//2) x (H-half = core%2); each core handles
512 rows x 1024 cols of one batch. The tiny grid is preprocessed on host into
per-row y-interpolated ramp-basis difference tables, so the device-side
z-interpolation is a dense sum of clamped ramps (no gathers):

  zeval(gz) = T0 + sum_{k=1..7} (T_k - T_{k-1}) * clamp01(gz - (k - 0.5))

which is exact for tent-weight trilinear slicing with clamped borders.
x-interpolation uses the 32px-block structure (grid-col pair constant per
block) with broadcast access patterns; the affine apply is done per pixel.
"""

import numpy as np
from contextlib import ExitStack

import concourse.bass as bass
import concourse.bacc as bacc
import concourse.tile as tile
from concourse import mybir
from concourse.bass_utils import run_bass_kernel_spmd

f32 = mybir.dt.float32
OP = mybir.AluOpType

# hardcoded problem shapes
B, C, GD, GH, GW = 4, 12, 8, 16, 16
H, W = 1024, 1024
NCORES = 8
RH = H // 2           # rows per core
NRG = RH // 128       # rowgroups per core
NM = W // 32          # 32px x-blocks across full W
XH = W // 2           # x half-width processed per inner step
MH = NM // 2          # x-blocks per half


def _ap(base, free):
    """AP with base's partition dim and explicit free dims."""
    return bass.AP(tensor=base.tensor, offset=base.offset, ap=[base.ap[0]] + free)


def _build_nc():
    nc = bacc.Bacc("TRN2", target_bir_lowering=False, debug=False,
                   num_devices=NCORES)

    guide_in = nc.dram_tensor("guide", [RH, W], f32, kind="ExternalInput")
    img_in = nc.dram_tensor("image", [3, RH, W], f32, kind="ExternalInput")
    tab_in = nc.dram_tensor("tab", [RH, C, 8, 2, 32], f32, kind="ExternalInput")
    wxr_in = nc.dram_tensor("wxr", [W], f32, kind="ExternalInput")
    out_t = nc.dram_tensor("out", [3, RH, W], f32, kind="ExternalOutput")

    with tile.TileContext(nc) as tc, ExitStack() as ctx:
        consts = ctx.enter_context(tc.tile_pool(name="consts", bufs=1))
        tabp = ctx.enter_context(tc.tile_pool(name="tabp", bufs=2))
        pix = ctx.enter_context(tc.tile_pool(name="pix", bufs=2))
        work = ctx.enter_context(tc.tile_pool(name="work", bufs=1))
        rpool = ctx.enter_context(tc.tile_pool(name="rpool", bufs=1))
        outp = ctx.enter_context(tc.tile_pool(name="outp", bufs=2))

        # one-time partition-broadcast constants
        wxr = consts.tile([128, W], f32)
        nc.sync.dma_start(
            out=wxr,
            in_=bass.AP(tensor=wxr_in[:].tensor, offset=wxr_in[:].offset,
                        ap=[[0, 128]] + wxr_in[:].ap),
        )

        for rg in range(NRG):
            r0 = rg * 128
            gt = pix.tile([128, W], f32, tag="guide")
            nc.sync.dma_start(out=gt, in_=guide_in[r0:r0 + 128, :])
            tabt = tabp.tile([128, C, 8, 2, 32], f32, tag="tab")
            nc.sync.dma_start(out=tabt, in_=tab_in[r0:r0 + 128])
            imgs = []
            for j in range(3):
                it = pix.tile([128, W], f32, tag=f"img{j}")
                nc.sync.dma_start(out=it, in_=img_in[j, r0:r0 + 128, :])
                imgs.append(it)

            for xh in range(2):
                x0 = xh * XH
                # ramps r_k = clamp01(8*guide - (k-0.5)), k=1..7 -> [128,7,XH]
                rr = rpool.tile([128, 7, XH], f32, tag="ramps")
                g2 = gt[:, x0:x0 + XH]
                for k in range(1, 8):
                    nc.vector.tensor_scalar(
                        out=rr[:, k - 1, :], in0=g2,
                        scalar1=8.0, scalar2=float(k) - 0.5,
                        op0=OP.mult, op1=OP.subtract,
                    )
                nc.vector.tensor_scalar(
                    out=rr, in0=rr, scalar1=0.0, scalar2=1.0,
                    op0=OP.max, op1=OP.min,
                )

                # channel groups: c0..7 on DVE, c8..11 on GPSIMD (~2x slower)
                groups = [(0, 8, nc.vector), (8, 4, nc.gpsimd)]
                coeffs = []
                for c0c, ng, eng in groups:
                    accB = work.tile([128, ng, MH, 32], f32, tag=f"accB{c0c}")
                    accD = work.tile([128, ng, MH, 32], f32, tag=f"accD{c0c}")
                    tmp = work.tile([128, ng, MH, 32], f32, tag=f"tmp{c0c}")

                    # init from k=0 slot (AL, AD), broadcast 32x along u (ACT)
                    for s, acc in ((0, accB), (1, accD)):
                        t5 = tabt[:, c0c:c0c + ng, 0, s, xh * MH:(xh + 1) * MH]
                        nc.scalar.copy(
                            out=acc, in_=_ap(t5, t5.ap[1:] + [[0, 32]])
                        )
                    for k in range(1, 8):
                        rk = rr[:, k - 1, :]
                        rk_b = _ap(rk, [[0, ng], [32, MH], [1, 32]])
                        for s, acc in ((0, accB), (1, accD)):
                            t5 = tabt[:, c0c:c0c + ng, k, s, xh * MH:(xh + 1) * MH]
                            eng.tensor_tensor(
                                out=tmp, in0=rk_b,
                                in1=_ap(t5, t5.ap[1:] + [[0, 32]]),
                                op=OP.mult,
                            )
                            eng.tensor_tensor(out=acc, in0=acc, in1=tmp,
                                              op=OP.add)
                    # coeff = accB + wxr * accD
                    w2 = wxr[:, x0:x0 + XH]
                    eng.tensor_tensor(
                        out=tmp, in0=accD,
                        in1=_ap(w2, [[0, ng], [32, MH], [1, 32]]),
                        op=OP.mult,
                    )
                    eng.tensor_tensor(out=accB, in0=accB, in1=tmp, op=OP.add)
                    coeffs.append(accB)

                # affine apply: out_i = c4i*R + c4i+1*G + c4i+2*B + c4i+3
                def cslice(c):
                    gidx = 0 if c < 8 else 1
                    return coeffs[gidx][:, c - 8 * gidx]  # [128, MH, 32]

                ims = [
                    _ap(imgs[j][:, x0:x0 + XH], [[32, MH], [1, 32]])
                    for j in range(3)
                ]
                for i in range(3):
                    eng = nc.vector if i < 2 else nc.gpsimd
                    t0 = work.tile([128, MH, 32], f32, tag=f"t0_{i}")
                    t1 = work.tile([128, MH, 32], f32, tag=f"t1_{i}")
                    cs = [cslice(4 * i + j) for j in range(4)]
                    eng.tensor_tensor(out=t0, in0=cs[0], in1=ims[0], op=OP.mult)
                    eng.tensor_tensor(out=t1, in0=cs[1], in1=ims[1], op=OP.mult)
                    eng.tensor_tensor(out=t0, in0=t0, in1=t1, op=OP.add)
                    eng.tensor_tensor(out=t1, in0=cs[2], in1=ims[2], op=OP.mult)
                    eng.tensor_tensor(out=t0, in0=t0, in1=t1, op=OP.add)
                    ot = outp.tile([128, MH, 32], f32, tag=f"o{i}")
                    eng.tensor_tensor(out=ot, in0=t0, in1=cs[3], op=OP.add)
                    nc.sync.dma_start(
                        out=out_t[i, r0:r0 + 128, x0:x0 + XH],
                        in_=_ap(ot[:, :], [[1, XH]]),
                    )
    nc.compile()
    return nc


_NC = None


def _get_nc():
    global _NC
    if _NC is None:
        _NC = _build_nc()
    return _NC


def _host_tables(grid_b, row0):
    """Build per-row ramp-basis tables [RH, C, 8, 2, 32] for rows
    [row0, row0+RH) of one batch's grid [C, GD, GH, GW]."""
    g = grid_b.astype(np.float32)
    y = np.arange(row0, row0 + RH, dtype=np.float64)
    gy = (y + 0.5) * GH / H
    fy = np.floor(gy - 0.5)
    wy1 = (gy - 0.5 - fy).astype(np.float32)
    iy0 = np.clip(fy, 0, GH - 1).astype(np.int64)
    iy1 = np.clip(fy + 1, 0, GH - 1).astype(np.int64)

    # F: [C, GD, RH, GW]
    F = (1.0 - wy1)[None, None, :, None] * g[:, :, iy0, :] \
        + wy1[None, None, :, None] * g[:, :, iy1, :]
    A = F[:, 0]                         # [C, RH, GW]
    D = F[:, 1:] - F[:, :-1]            # [C, 7, RH, GW]

    x = np.arange(W, dtype=np.float64)
    gx = (x + 0.5) * GW / W
    fx = np.floor(gx - 0.5)
    ix0 = np.clip(fx, 0, GW - 1).astype(np.int64).reshape(NM, 32)[:, 0]
    ix1 = np.clip(fx + 1, 0, GW - 1).astype(np.int64).reshape(NM, 32)[:, 0]

    tab = np.empty((RH, C, 8, 2, 32), np.float32)
    AL, AR = A[:, :, ix0], A[:, :, ix1]              # [C, RH, NM]
    tab[:, :, 0, 0, :] = AL.transpose(1, 0, 2)
    tab[:, :, 0, 1, :] = (AR - AL).transpose(1, 0, 2)
    DL, DR = D[:, :, :, ix0], D[:, :, :, ix1]        # [C, 7, RH, NM]
    tab[:, :, 1:, 0, :] = DL.transpose(2, 0, 1, 3)
    tab[:, :, 1:, 1, :] = (DR - DL).transpose(2, 0, 1, 3)
    return np.ascontiguousarray(tab)


def _host_wxr():
    x = np.arange(W, dtype=np.float64)
    gx = (x + 0.5) * GW / W
    fx = np.floor(gx - 0.5)
    return (gx - 0.5 - fx).astype(np.float32)


def _make_in_maps(grid, guide, image):
    wxr = _host_wxr()
    in_maps = []
    for core in range(NCORES):
        b, hh = core // 2, core % 2
        r0 = hh * RH
        in_maps.append({
            "guide": np.ascontiguousarray(guide[b, r0:r0 + RH]).astype(np.float32),
            "image": np.ascontiguousarray(image[b, :, r0:r0 + RH]).astype(np.float32),
            "tab": _host_tables(grid[b], r0),
            "wxr": wxr,
        })
    return in_maps


def _run(grid, guide, image, trace=False):
    nc = _get_nc()
    in_maps = _make_in_maps(grid, guide, image)
    res = run_bass_kernel_spmd(nc, in_maps, core_ids=list(range(NCORES)),
                               trace=trace)
    out = np.empty((B, 3, H, W), np.float32)
    for core in range(NCORES):
        b, hh = core // 2, core % 2
        out[b, :, hh * RH:hh * RH + RH, :] = res.results[core]["out"]
    return out, res


def kernel(grid, guide, image):
    grid = np.asarray(grid, dtype=np.float32)
    guide = np.asarray(guide, dtype=np.float32)
    image = np.asarray(image, dtype=np.float32)
    out, _ = _run(grid, guide, image, trace=False)
    return out


# revision 24
# speedup vs baseline: 1.0331x; 1.0022x over previous
"""Bilateral slice-apply kernel for Trainium2 (8 NeuronCores, SPMD).

Sharding: core = (batch b = core//2) x (H-half = core%2); each core handles
512 rows x 1024 cols of one batch. The tiny grid is preprocessed on host into
per-row y-interpolated ramp-basis difference tables, so the device-side
z-interpolation is a dense sum of clamped ramps (no gathers):

  zeval(gz) = T0 + sum_{k=1..7} (T_k - T_{k-1}) * clamp01(gz - (k - 0.5))

which is exact for tent-weight trilinear slicing with clamped borders.
x-interpolation uses the 32px-block structure (grid-col pair constant per
block) with broadcast access patterns; the affine apply is done per pixel.
"""

import numpy as np
from contextlib import ExitStack

import concourse.bass as bass
import concourse.bacc as bacc
import concourse.tile as tile
from concourse import mybir
from concourse.bass_utils import run_bass_kernel_spmd

f32 = mybir.dt.float32
OP = mybir.AluOpType

# hardcoded problem shapes
B, C, GD, GH, GW = 4, 12, 8, 16, 16
H, W = 1024, 1024
NCORES = 8
RH = H // 2           # rows per core
NRG = RH // 128       # rowgroups per core
NM = W // 32          # 32px x-blocks across full W
XH = W // 2           # x half-width processed per inner step
MH = NM // 2          # x-blocks per half


def _ap(base, free):
    """AP with base's partition dim and explicit free dims."""
    return bass.AP(tensor=base.tensor, offset=base.offset, ap=[base.ap[0]] + free)


def _build_nc():
    nc = bacc.Bacc("TRN2", target_bir_lowering=False, debug=False,
                   num_devices=NCORES)

    guide_in = nc.dram_tensor("guide", [RH, W], f32, kind="ExternalInput")
    img_in = nc.dram_tensor("image", [3, RH, W], f32, kind="ExternalInput")
    tab_in = nc.dram_tensor("tab", [RH, C, 8, 2, 32], f32, kind="ExternalInput")
    wxr_in = nc.dram_tensor("wxr", [W], f32, kind="ExternalInput")
    out_t = nc.dram_tensor("out", [3, RH, W], f32, kind="ExternalOutput")

    with tile.TileContext(nc) as tc, ExitStack() as ctx:
        consts = ctx.enter_context(tc.tile_pool(name="consts", bufs=1))
        tabp = ctx.enter_context(tc.tile_pool(name="tabp", bufs=2))
        pix = ctx.enter_context(tc.tile_pool(name="pix", bufs=2))
        work = ctx.enter_context(tc.tile_pool(name="work", bufs=1))
        rpool = ctx.enter_context(tc.tile_pool(name="rpool", bufs=1))
        outp = ctx.enter_context(tc.tile_pool(name="outp", bufs=2))

        # one-time partition-broadcast constants
        wxr = consts.tile([128, W], f32)
        nc.sync.dma_start(
            out=wxr,
            in_=bass.AP(tensor=wxr_in[:].tensor, offset=wxr_in[:].offset,
                        ap=[[0, 128]] + wxr_in[:].ap),
        )

        for rg in range(NRG):
            r0 = rg * 128
            gt = pix.tile([128, W], f32, tag="guide")
            nc.sync.dma_start(out=gt, in_=guide_in[r0:r0 + 128, :])
            tabt = tabp.tile([128, C, 8, 2, 32], f32, tag="tab")
            nc.sync.dma_start(out=tabt, in_=tab_in[r0:r0 + 128])
            imgs = []
            for j in range(3):
                it = pix.tile([128, W], f32, tag=f"img{j}")
                nc.sync.dma_start(out=it, in_=img_in[j, r0:r0 + 128, :])
                imgs.append(it)

            for xh in range(2):
                x0 = xh * XH
                # ramps r_k = clamp01(8*guide - (k-0.5)), k=1..7 -> [128,7,XH]
                rr = rpool.tile([128, 7, XH], f32, tag="ramps")
                g2 = gt[:, x0:x0 + XH]
                for k in range(1, 8):
                    nc.vector.tensor_scalar(
                        out=rr[:, k - 1, :], in0=g2,
                        scalar1=8.0, scalar2=float(k) - 0.5,
                        op0=OP.mult, op1=OP.subtract,
                    )
                nc.vector.tensor_scalar(
                    out=rr, in0=rr, scalar1=0.0, scalar2=1.0,
                    op0=OP.max, op1=OP.min,
                )

                # channel groups: c0..7 on DVE, c8..11 on GPSIMD (~2x slower)
                groups = [(0, 8, nc.vector), (8, 4, nc.gpsimd)]
                coeffs = []
                for c0c, ng, eng in groups:
                    accB = work.tile([128, ng, MH, 32], f32, tag=f"accB{c0c}")
                    accD = work.tile([128, ng, MH, 32], f32, tag=f"accD{c0c}")
                    tmp = work.tile([128, ng, MH, 32], f32, tag=f"tmp{c0c}")

                    # init from k=0 slot (AL, AD), broadcast 32x along u (ACT)
                    for s, acc in ((0, accB), (1, accD)):
                        t5 = tabt[:, c0c:c0c + ng, 0, s, xh * MH:(xh + 1) * MH]
                        nc.scalar.copy(
                            out=acc, in_=_ap(t5, t5.ap[1:] + [[0, 32]])
                        )
                    for k in range(1, 8):
                        rk = rr[:, k - 1, :]
                        rk_b = _ap(rk, [[0, ng], [32, MH], [1, 32]])
                        for s, acc in ((0, accB), (1, accD)):
                            t5 = tabt[:, c0c:c0c + ng, k, s, xh * MH:(xh + 1) * MH]
                            eng.tensor_tensor(
                                out=tmp, in0=rk_b,
                                in1=_ap(t5, t5.ap[1:] + [[0, 32]]),
                                op=OP.mult,
                            )
                            eng.tensor_tensor(out=acc, in0=acc, in1=tmp,
                                              op=OP.add)
                    # coeff = accB + wxr * accD
                    w2 = wxr[:, x0:x0 + XH]
                    eng.tensor_tensor(
                        out=tmp, in0=accD,
                        in1=_ap(w2, [[0, ng], [32, MH], [1, 32]]),
                        op=OP.mult,
                    )
                    eng.tensor_tensor(out=accB, in0=accB, in1=tmp, op=OP.add)
                    coeffs.append(accB)

                # affine apply: out_i = c4i*R + c4i+1*G + c4i+2*B + c4i+3
                def cslice(c):
                    gidx = 0 if c < 8 else 1
                    return coeffs[gidx][:, c - 8 * gidx]  # [128, MH, 32]

                ims = [
                    _ap(imgs[j][:, x0:x0 + XH], [[32, MH], [1, 32]])
                    for j in range(3)
                ]
                # i=0,1 fused (coeff channels {j, 4+j} stride-4 in the DVE
                # group tile); i=2 separately on GPSIMD from its own group
                t0 = work.tile([128, 2, MH, 32], f32, tag="t0_01")
                t1 = work.tile([128, 2, MH, 32], f32, tag="t1_01")
                c01 = coeffs[0]  # [128, 8, MH, 32], channels 0..7

                def cpair(j):  # channels {j, 4+j} as [128, 2, MH, 32]
                    a = c01[:, j]
                    return _ap(a, [[4 * MH * 32, 2]] + a.ap[1:])

                im2 = [_ap(imgs[j][:, x0:x0 + XH],
                           [[0, 2], [32, MH], [1, 32]]) for j in range(3)]
                nc.vector.tensor_tensor(out=t0, in0=cpair(0), in1=im2[0],
                                        op=OP.mult)
                nc.vector.tensor_tensor(out=t1, in0=cpair(1), in1=im2[1],
                                        op=OP.mult)
                nc.vector.tensor_tensor(out=t0, in0=t0, in1=t1, op=OP.add)
                nc.vector.tensor_tensor(out=t1, in0=cpair(2), in1=im2[2],
                                        op=OP.mult)
                nc.vector.tensor_tensor(out=t0, in0=t0, in1=t1, op=OP.add)
                ot01 = outp.tile([128, 2, MH, 32], f32, tag="o01")
                nc.vector.tensor_tensor(out=ot01, in0=t0, in1=cpair(3),
                                        op=OP.add)
                o2d = out_t[0, r0:r0 + 128, x0:x0 + XH]
                nc.sync.dma_start(
                    out=bass.AP(tensor=o2d.tensor, offset=o2d.offset,
                                ap=[o2d.ap[0], [RH * W, 2], o2d.ap[1]]),
                    in_=_ap(ot01[:, :], [ot01[:, :].ap[1]] + [[1, XH]]),
                )
                t0g = work.tile([128, MH, 32], f32, tag="t0_2")
                t1g = work.tile([128, MH, 32], f32, tag="t1_2")
                cs = [cslice(8 + j) for j in range(4)]
                nc.gpsimd.tensor_tensor(out=t0g, in0=cs[0], in1=ims[0],
                                        op=OP.mult)
                nc.gpsimd.tensor_tensor(out=t1g, in0=cs[1], in1=ims[1],
                                        op=OP.mult)
                nc.gpsimd.tensor_tensor(out=t0g, in0=t0g, in1=t1g, op=OP.add)
                nc.gpsimd.tensor_tensor(out=t1g, in0=cs[2], in1=ims[2],
                                        op=OP.mult)
                nc.gpsimd.tensor_tensor(out=t0g, in0=t0g, in1=t1g, op=OP.add)
                ot2 = outp.tile([128, MH, 32], f32, tag="o2")
                nc.gpsimd.tensor_tensor(out=ot2, in0=t0g, in1=cs[3], op=OP.add)
                nc.sync.dma_start(
                    out=out_t[2, r0:r0 + 128, x0:x0 + XH],
                    in_=_ap(ot2[:, :], [[1, XH]]),
                )
    nc.compile()
    return nc


_NC = None


def _get_nc():
    global _NC
    if _NC is None:
        _NC = _build_nc()
    return _NC


def _host_tables(grid_b, row0):
    """Build per-row ramp-basis tables [RH, C, 8, 2, 32] for rows
    [row0, row0+RH) of one batch's grid [C, GD, GH, GW]."""
    g = grid_b.astype(np.float32)
    y = np.arange(row0, row0 + RH, dtype=np.float64)
    gy = (y + 0.5) * GH / H
    fy = np.floor(gy - 0.5)
    wy1 = (gy - 0.5 - fy).astype(np.float32)
    iy0 = np.clip(fy, 0, GH - 1).astype(np.int64)
    iy1 = np.clip(fy + 1, 0, GH - 1).astype(np.int64)

    # F: [C, GD, RH, GW]
    F = (1.0 - wy1)[None, None, :, None] * g[:, :, iy0, :] \
        + wy1[None, None, :, None] * g[:, :, iy1, :]
    A = F[:, 0]                         # [C, RH, GW]
    D = F[:, 1:] - F[:, :-1]            # [C, 7, RH, GW]

    x = np.arange(W, dtype=np.float64)
    gx = (x + 0.5) * GW / W
    fx = np.floor(gx - 0.5)
    ix0 = np.clip(fx, 0, GW - 1).astype(np.int64).reshape(NM, 32)[:, 0]
    ix1 = np.clip(fx + 1, 0, GW - 1).astype(np.int64).reshape(NM, 32)[:, 0]

    tab = np.empty((RH, C, 8, 2, 32), np.float32)
    AL, AR = A[:, :, ix0], A[:, :, ix1]              # [C, RH, NM]
    tab[:, :, 0, 0, :] = AL.transpose(1, 0, 2)
    tab[:, :, 0, 1, :] = (AR - AL).transpose(1, 0, 2)
    DL, DR = D[:, :, :, ix0], D[:, :, :, ix1]        # [C, 7, RH, NM]
    tab[:, :, 1:, 0, :] = DL.transpose(2, 0, 1, 3)
    tab[:, :, 1:, 1, :] = (DR - DL).transpose(2, 0, 1, 3)
    return np.ascontiguousarray(tab)


def _host_wxr():
    x = np.arange(W, dtype=np.float64)
    gx = (x + 0.5) * GW / W
    fx = np.floor(gx - 0.5)
    return (gx - 0.5 - fx).astype(np.float32)


def _make_in_maps(grid, guide, image):
    wxr = _host_wxr()
    in_maps = []
    for core in range(NCORES):
        b, hh = core // 2, core % 2
        r0 = hh * RH
        in_maps.append({
            "guide": np.ascontiguousarray(guide[b, r0:r0 + RH]).astype(np.float32),
            "image": np.ascontiguousarray(image[b, :, r0:r0 + RH]).astype(np.float32),
            "tab": _host_tables(grid[b], r0),
            "wxr": wxr,
        })
    return in_maps


def _run(grid, guide, image, trace=False):
    nc = _get_nc()
    in_maps = _make_in_maps(grid, guide, image)
    res = run_bass_kernel_spmd(nc, in_maps, core_ids=list(range(NCORES)),
                               trace=trace)
    out = np.empty((B, 3, H, W), np.float32)
    for core in range(NCORES):
        b, hh = core // 2, core % 2
        out[b, :, hh * RH:hh * RH + RH, :] = res.results[core]["out"]
    return out, res


def kernel(grid, guide, image):
    grid = np.asarray(grid, dtype=np.float32)
    guide = np.asarray(guide, dtype=np.float32)
    image = np.asarray(image, dtype=np.float32)
    out, _ = _run(grid, guide, image, trace=False)
    return out


# revision 26
# speedup vs baseline: 1.0488x; 1.0152x over previous
"""Bilateral slice-apply kernel for Trainium2 (8 NeuronCores, SPMD).

Sharding: core = (batch b = core//2) x (H-half = core%2); each core handles
512 rows x 1024 cols of one batch. The tiny grid is preprocessed on host into
per-row y-interpolated ramp-basis difference tables, so the device-side
z-interpolation is a dense sum of clamped ramps (no gathers):

  zeval(gz) = T0 + sum_{k=1..7} (T_k - T_{k-1}) * clamp01(gz - (k - 0.5))

which is exact for tent-weight trilinear slicing with clamped borders.
x-interpolation uses the 32px-block structure (grid-col pair constant per
block) with broadcast access patterns; the affine apply is done per pixel.
"""

import numpy as np
from contextlib import ExitStack

import concourse.bass as bass
import concourse.bacc as bacc
import concourse.tile as tile
from concourse import mybir
from concourse.bass_utils import run_bass_kernel_spmd

f32 = mybir.dt.float32
OP = mybir.AluOpType

# hardcoded problem shapes
B, C, GD, GH, GW = 4, 12, 8, 16, 16
H, W = 1024, 1024
NCORES = 8
RH = H // 2           # rows per core
NRG = RH // 128       # rowgroups per core
NM = W // 32          # 32px x-blocks across full W
XH = W // 2           # x half-width processed per inner step
MH = NM // 2          # x-blocks per half


def _ap(base, free):
    """AP with base's partition dim and explicit free dims."""
    return bass.AP(tensor=base.tensor, offset=base.offset, ap=[base.ap[0]] + free)


def _build_nc():
    nc = bacc.Bacc("TRN2", target_bir_lowering=False, debug=False,
                   num_devices=NCORES)

    guide_in = nc.dram_tensor("guide", [RH, W], f32, kind="ExternalInput")
    img_in = nc.dram_tensor("image", [3, RH, W], f32, kind="ExternalInput")
    tab_in = nc.dram_tensor("tab", [RH, 8, C, 2, 32], f32, kind="ExternalInput")
    wxr_in = nc.dram_tensor("wxr", [W], f32, kind="ExternalInput")
    out_t = nc.dram_tensor("out", [3, RH, W], f32, kind="ExternalOutput")

    with tile.TileContext(nc) as tc, ExitStack() as ctx:
        consts = ctx.enter_context(tc.tile_pool(name="consts", bufs=1))
        tabp = ctx.enter_context(tc.tile_pool(name="tabp", bufs=2))
        pix = ctx.enter_context(tc.tile_pool(name="pix", bufs=2))
        work = ctx.enter_context(tc.tile_pool(name="work", bufs=1))
        rpool = ctx.enter_context(tc.tile_pool(name="rpool", bufs=1))

        # one-time partition-broadcast constants
        wxr = consts.tile([128, W], f32)
        nc.sync.dma_start(
            out=wxr,
            in_=bass.AP(tensor=wxr_in[:].tensor, offset=wxr_in[:].offset,
                        ap=[[0, 128]] + wxr_in[:].ap),
        )

        for rg in range(NRG):
            r0 = rg * 128
            gt = pix.tile([128, W], f32, tag="guide")
            nc.sync.dma_start(out=gt, in_=guide_in[r0:r0 + 128, :])
            tabt = tabp.tile([128, 8, C, 2, 32], f32, tag="tab")
            nc.sync.dma_start(out=tabt, in_=tab_in[r0:r0 + 128])
            imgs = []
            for j in range(3):
                it = pix.tile([128, W], f32, tag=f"img{j}")
                nc.sync.dma_start(out=it, in_=img_in[j, r0:r0 + 128, :])
                imgs.append(it)

            for xh in range(2):
                x0 = xh * XH
                # ramps r_k = clamp01(8*guide - (k-0.5)), k=1..7 -> [128,7,XH]
                rr = rpool.tile([128, 7, XH], f32, tag="ramps")
                g2 = gt[:, x0:x0 + XH]
                for k in range(1, 8):
                    nc.vector.tensor_scalar(
                        out=rr[:, k - 1, :], in0=g2,
                        scalar1=8.0, scalar2=float(k) - 0.5,
                        op0=OP.mult, op1=OP.subtract,
                    )
                nc.vector.tensor_scalar(
                    out=rr, in0=rr, scalar1=0.0, scalar2=1.0,
                    op0=OP.max, op1=OP.min,
                )

                # channel groups: c0..7 on DVE, c8..11 on GPSIMD (~2x slower)
                groups = [(0, 8, nc.vector), (8, 4, nc.gpsimd)]
                coeffs = []
                for c0c, ng, eng in groups:
                    # acc holds (channel, side) interleaved: cs = 2c+s, B even
                    acc = work.tile([128, 2 * ng, MH, 32], f32, tag=f"acc{c0c}")
                    tmp = work.tile([128, 2 * ng, MH, 32], f32, tag=f"tmp{c0c}")

                    def tslice(k):
                        # fused (c,s) table read: c-stride 64 = 2 x s-stride 32
                        t5 = tabt[:, k, c0c:c0c + ng, 0:2,
                                  xh * MH:(xh + 1) * MH]
                        return _ap(t5, [[32, 2 * ng], [1, MH], [0, 32]])

                    # init from k=0 slot (A-tables), broadcast 32x along u
                    nc.scalar.copy(out=acc, in_=tslice(0))
                    for k in range(1, 8):
                        rk = rr[:, k - 1, :]
                        rk_b = _ap(rk, [[0, 2 * ng], [32, MH], [1, 32]])
                        eng.tensor_tensor(out=tmp, in0=rk_b, in1=tslice(k),
                                          op=OP.mult)
                        eng.tensor_tensor(out=acc, in0=acc, in1=tmp, op=OP.add)
                    # coeff_c = acc[2c] + wxr * acc[2c+1]
                    w2 = wxr[:, x0:x0 + XH]
                    CS = MH * 32
                    accD_ap = _ap(acc[:, 1], [[2 * CS, ng], [32, MH], [1, 32]])
                    accB_ap = _ap(acc[:, 0], [[2 * CS, ng], [32, MH], [1, 32]])
                    tmpd = tmp[:, 0:ng]
                    eng.tensor_tensor(
                        out=tmpd, in0=accD_ap,
                        in1=_ap(w2, [[0, ng], [32, MH], [1, 32]]),
                        op=OP.mult,
                    )
                    eng.tensor_tensor(out=accB_ap, in0=accB_ap, in1=tmpd,
                                      op=OP.add)
                    coeffs.append(acc)

                # affine apply: out_i = c4i*R + c4i+1*G + c4i+2*B + c4i+3
                def cslice(c):
                    gidx = 0 if c < 8 else 1
                    return coeffs[gidx][:, 2 * (c - 8 * gidx)]  # [128, MH, 32]

                ims = [
                    _ap(imgs[j][:, x0:x0 + XH], [[32, MH], [1, 32]])
                    for j in range(3)
                ]
                # i=0,1 fused (coeff channels {j, 4+j} stride-4 in the DVE
                # group tile); i=2 separately on GPSIMD from its own group
                t0 = work.tile([128, 2, MH, 32], f32, tag="t0_01")
                t1 = work.tile([128, 2, MH, 32], f32, tag="t1_01")
                c01 = coeffs[0]  # [128, 8, MH, 32], channels 0..7

                def cpair(j):  # channels {j, 4+j} as [128, 2, MH, 32]
                    a = c01[:, 2 * j]
                    return _ap(a, [[8 * MH * 32, 2]] + a.ap[1:])

                im2 = [_ap(imgs[j][:, x0:x0 + XH],
                           [[0, 2], [32, MH], [1, 32]]) for j in range(3)]
                nc.vector.tensor_tensor(out=t0, in0=cpair(0), in1=im2[0],
                                        op=OP.mult)
                nc.vector.tensor_tensor(out=t1, in0=cpair(1), in1=im2[1],
                                        op=OP.mult)
                nc.vector.tensor_tensor(out=t0, in0=t0, in1=t1, op=OP.add)
                nc.vector.tensor_tensor(out=t1, in0=cpair(2), in1=im2[2],
                                        op=OP.mult)
                nc.vector.tensor_tensor(out=t0, in0=t0, in1=t1, op=OP.add)
                ot01 = t1
                nc.vector.tensor_tensor(out=ot01, in0=t0, in1=cpair(3),
                                        op=OP.add)
                o2d = out_t[0, r0:r0 + 128, x0:x0 + XH]
                nc.sync.dma_start(
                    out=bass.AP(tensor=o2d.tensor, offset=o2d.offset,
                                ap=[o2d.ap[0], [RH * W, 2], o2d.ap[1]]),
                    in_=_ap(ot01[:, :], [ot01[:, :].ap[1]] + [[1, XH]]),
                )
                t0g = work.tile([128, MH, 32], f32, tag="t0_2")
                t1g = work.tile([128, MH, 32], f32, tag="t1_2")
                cs = [cslice(8 + j) for j in range(4)]
                nc.gpsimd.tensor_tensor(out=t0g, in0=cs[0], in1=ims[0],
                                        op=OP.mult)
                nc.gpsimd.tensor_tensor(out=t1g, in0=cs[1], in1=ims[1],
                                        op=OP.mult)
                nc.gpsimd.tensor_tensor(out=t0g, in0=t0g, in1=t1g, op=OP.add)
                nc.gpsimd.tensor_tensor(out=t1g, in0=cs[2], in1=ims[2],
                                        op=OP.mult)
                nc.gpsimd.tensor_tensor(out=t0g, in0=t0g, in1=t1g, op=OP.add)
                ot2 = t1g
                nc.gpsimd.tensor_tensor(out=ot2, in0=t0g, in1=cs[3], op=OP.add)
                nc.sync.dma_start(
                    out=out_t[2, r0:r0 + 128, x0:x0 + XH],
                    in_=_ap(ot2[:, :], [[1, XH]]),
                )
    nc.compile()
    return nc


_NC = None


def _get_nc():
    global _NC
    if _NC is None:
        _NC = _build_nc()
    return _NC


def _host_tables(grid_b, row0):
    """Build per-row ramp-basis tables [RH, C, 8, 2, 32] for rows
    [row0, row0+RH) of one batch's grid [C, GD, GH, GW]."""
    g = grid_b.astype(np.float32)
    y = np.arange(row0, row0 + RH, dtype=np.float64)
    gy = (y + 0.5) * GH / H
    fy = np.floor(gy - 0.5)
    wy1 = (gy - 0.5 - fy).astype(np.float32)
    iy0 = np.clip(fy, 0, GH - 1).astype(np.int64)
    iy1 = np.clip(fy + 1, 0, GH - 1).astype(np.int64)

    # F: [C, GD, RH, GW]
    F = (1.0 - wy1)[None, None, :, None] * g[:, :, iy0, :] \
        + wy1[None, None, :, None] * g[:, :, iy1, :]
    A = F[:, 0]                         # [C, RH, GW]
    D = F[:, 1:] - F[:, :-1]            # [C, 7, RH, GW]

    x = np.arange(W, dtype=np.float64)
    gx = (x + 0.5) * GW / W
    fx = np.floor(gx - 0.5)
    ix0 = np.clip(fx, 0, GW - 1).astype(np.int64).reshape(NM, 32)[:, 0]
    ix1 = np.clip(fx + 1, 0, GW - 1).astype(np.int64).reshape(NM, 32)[:, 0]

    tab = np.empty((RH, 8, C, 2, 32), np.float32)
    AL, AR = A[:, :, ix0], A[:, :, ix1]              # [C, RH, NM]
    tab[:, 0, :, 0, :] = AL.transpose(1, 0, 2)
    tab[:, 0, :, 1, :] = (AR - AL).transpose(1, 0, 2)
    DL, DR = D[:, :, :, ix0], D[:, :, :, ix1]        # [C, 7, RH, NM]
    tab[:, 1:, :, 0, :] = DL.transpose(2, 1, 0, 3)
    tab[:, 1:, :, 1, :] = (DR - DL).transpose(2, 1, 0, 3)
    return np.ascontiguousarray(tab)


def _host_wxr():
    x = np.arange(W, dtype=np.float64)
    gx = (x + 0.5) * GW / W
    fx = np.floor(gx - 0.5)
    return (gx - 0.5 - fx).astype(np.float32)


def _make_in_maps(grid, guide, image):
    wxr = _host_wxr()
    in_maps = []
    for core in range(NCORES):
        b, hh = core // 2, core % 2
        r0 = hh * RH
        in_maps.append({
            "guide": np.ascontiguousarray(guide[b, r0:r0 + RH]).astype(np.float32),
            "image": np.ascontiguousarray(image[b, :, r0:r0 + RH]).astype(np.float32),
            "tab": _host_tables(grid[b], r0),
            "wxr": wxr,
        })
    return in_maps


def _run(grid, guide, image, trace=False):
    nc = _get_nc()
    in_maps = _make_in_maps(grid, guide, image)
    res = run_bass_kernel_spmd(nc, in_maps, core_ids=list(range(NCORES)),
                               trace=trace)
    out = np.empty((B, 3, H, W), np.float32)
    for core in range(NCORES):
        b, hh = core // 2, core % 2
        out[b, :, hh * RH:hh * RH + RH, :] = res.results[core]["out"]
    return out, res


def kernel(grid, guide, image):
    grid = np.asarray(grid, dtype=np.float32)
    guide = np.asarray(guide, dtype=np.float32)
    image = np.asarray(image, dtype=np.float32)
    out, _ = _run(grid, guide, image, trace=False)
    return out
